# revision 1
# baseline (speedup 1.0000x reference)
"""3x3 morphological erosion (min-pool, stride 1, padding 1e9) on Trainium2.

Contract: kernel(x, m) takes the FULL inputs (x: (8, 8, 1024, 1024) float32,
m == 1) and returns the full erosion output. Internally the batch dim is
sharded across 8 NeuronCores (pure data parallel); each core runs the same
Bass/Tile kernel on its (8, 1024, 1024) shard via a shard_map'd PJRT call.

Per-core kernel layout (all compute partition-aligned; the BIR verifier in
this toolchain rejects partition-shifted compute operands, and walrus here
allows at most ONE sync wait per instruction — see _split_sync_waits):
  partition p holds K=8 consecutive image rows as free-dim segments of a
  (channel, W-half) tile. The horizontal 3-tap min is two free-dim-shifted
  tensor_tensor(min) ops (h1, h3); the vertical 3-tap min is segment-shifted
  ops plus two partition-shifted SBUF->SBUF DMA copies:
    H3X[p] = h3 of next partition's first row  (completes v1 of row K*p+K-1)
    V1B[p] = v1 of prev partition's last row   (completes out of row K*p)
  Pad columns / pad partitions hold 1e9 and are written once per pool slot;
  no per-tile memsets are needed.
"""

import numpy as np

import concourse.bass as bass
import concourse.mybir as mybir
from concourse.tile import TileContext

F32 = mybir.dt.float32
MIN = mybir.AluOpType.min
PAD = 1.0e9

N_CORES = 8


def _split_sync_waits(nc, limit=1):
    """walrus in this container rejects instructions carrying more than
    `limit` sync waits ("Too many sync wait commands"). Move excess waits
    onto NOPs inserted just before the offending instruction on the same
    engine — semantically identical (the engine blocks on each wait in
    order before executing the instruction)."""
    seq = [0]
    for f in nc.m.functions:
        for b in f.blocks:
            lst = b.instructions
            i = 0
            while i < len(lst):
                ins = lst[i]
                si = ins.sync_info
                nadd = 0
                if si is not None and len(si.on_wait) > limit:
                    waits = list(si.on_wait)
                    keep, extra = waits[:limit], waits[limit:]
                    nops = []
                    while extra:
                        chunk, extra = extra[:limit], extra[limit:]
                        nop = mybir.InstNoOp(name=f"WSPLIT-{seq[0]}", ins=[], outs=[])
                        seq[0] += 1
                        nop.engine = ins.engine
                        nop.sync_info = mybir.SyncInfo(on_wait=chunk, on_update=[])
                        nops.append(nop)
                    ins.sync_info = mybir.SyncInfo(on_wait=keep, on_update=list(si.on_update))
                    for j, nop in enumerate(nops):
                        lst.insert(i + j, nop)
                        try:
                            nc.register_instruction(nop, overwrite=True)
                        except Exception:
                            pass
                    nadd = len(nops)
                i += nadd + 1


def _build_erosion(C=8, H=1024, W=1024, K=8, x_bufs=2, h1_bufs=2, h3_bufs=2,
                   v1_bufs=2, out_bufs=2, sb_bufs=2):
    assert H % K == 0
    P = H // K            # partitions per tile (128 at full scale)
    Wh = W // 2           # half width per tile
    SW = Wh + 2           # X segment width (1 pad col each side)
    HW1 = Wh + 1          # H1 segment width

    nc = bass.Bass()
    x = nc.dram_tensor("x", [C, H, W], F32, kind="ExternalInput")
    y = nc.dram_tensor("y", [C, H, W], F32, kind="ExternalOutput")

    with TileContext(nc) as tc:
        with (
            tc.tile_pool(name="xl", bufs=x_bufs) as xl_pool,
            tc.tile_pool(name="xr", bufs=x_bufs) as xr_pool,
            tc.tile_pool(name="h1p", bufs=h1_bufs) as h1_pool,
            tc.tile_pool(name="h3p", bufs=h3_bufs) as h3_pool,
            tc.tile_pool(name="v1p", bufs=v1_bufs) as v1_pool,
            tc.tile_pool(name="outp", bufs=out_bufs) as out_pool,
            tc.tile_pool(name="h3x", bufs=sb_bufs) as h3x_pool,
            tc.tile_pool(name="v1b", bufs=sb_bufs) as v1b_pool,
        ):
            xl_slots = [xl_pool.tile([P, K * SW], F32, tag="xl", name=f"XL{i}") for i in range(x_bufs)]
            xr_slots = [xr_pool.tile([P, K * SW], F32, tag="xr", name=f"XR{i}") for i in range(x_bufs)]
            h3x_slots = [h3x_pool.tile([P, Wh], F32, tag="h3x", name=f"H3X{i}") for i in range(sb_bufs)]
            v1b_slots = [v1b_pool.tile([P, Wh], F32, tag="v1b", name=f"V1B{i}") for i in range(sb_bufs)]
            for s in xl_slots:
                s3 = s[:, :].rearrange("p (n c) -> p n c", c=SW)
                nc.vector.memset(s3[:, :, 0:1], PAD)
            for s in xr_slots:
                s3 = s[:, :].rearrange("p (n c) -> p n c", c=SW)
                nc.vector.memset(s3[:, :, SW - 1:SW], PAD)
            for s in h3x_slots:
                nc.vector.memset(s[:, :], PAD)
            for s in v1b_slots:
                nc.vector.memset(s[:, :], PAD)

            idx = [0, 0, 0]

            for c in range(C):
                for side in (0, 1):
                    if side == 0:
                        X = xl_slots[idx[0] % x_bufs]; idx[0] += 1
                        src = x[c].rearrange("(p k) w -> p k w", k=K)[:, :, 0:Wh + 1]
                        dst = X[:, :].rearrange("p (n c) -> p n c", c=SW)[:, :, 1:SW]
                    else:
                        X = xr_slots[idx[1] % x_bufs]; idx[1] += 1
                        src = x[c].rearrange("(p k) w -> p k w", k=K)[:, :, Wh - 1:W]
                        dst = X[:, :].rearrange("p (n c) -> p n c", c=SW)[:, :, 0:SW - 1]
                    nc.sync.dma_start(out=dst, in_=src)

                    x3 = X[:, :].rearrange("p (n c) -> p n c", c=SW)
                    H1 = h1_pool.tile([P, K * HW1], F32, tag="h1", name=f"H1_{c}_{side}")
                    h13 = H1[:, :].rearrange("p (n c) -> p n c", c=HW1)
                    nc.vector.tensor_tensor(out=h13[:, :, :], in0=x3[:, :, 0:SW - 1],
                                            in1=x3[:, :, 1:SW], op=MIN)

                    H3 = h3_pool.tile([P, K * Wh], F32, tag="h3", name=f"H3_{c}_{side}")
                    h33 = H3[:, :].rearrange("p (n c) -> p n c", c=Wh)
                    nc.vector.tensor_tensor(out=h33[:, :, :], in0=h13[:, :, 0:Wh],
                                            in1=h13[:, :, 1:HW1], op=MIN)

                    H3X = h3x_slots[idx[2] % sb_bufs]
                    V1B = v1b_slots[idx[2] % sb_bufs]; idx[2] += 1
                    nc.scalar.dma_start(out=H3X[0:P - 1, :], in_=H3[1:P, 0:Wh])

                    V1 = v1_pool.tile([P, K * Wh], F32, tag="v1", name=f"V1_{c}_{side}")
                    nc.vector.tensor_tensor(out=V1[:, 0:(K - 1) * Wh], in0=H3[:, 0:(K - 1) * Wh],
                                            in1=H3[:, Wh:K * Wh], op=MIN)
                    nc.vector.tensor_tensor(out=V1[:, (K - 1) * Wh:K * Wh],
                                            in0=H3[:, (K - 1) * Wh:K * Wh], in1=H3X[:, :], op=MIN)

                    nc.scalar.dma_start(out=V1B[1:P, :], in_=V1[0:P - 1, (K - 1) * Wh:K * Wh])

                    OUT = out_pool.tile([P, K * Wh], F32, tag="out", name=f"OUT_{c}_{side}")
                    nc.vector.tensor_tensor(out=OUT[:, Wh:K * Wh], in0=V1[:, 0:(K - 1) * Wh],
                                            in1=V1[:, Wh:K * Wh], op=MIN)
                    nc.vector.tensor_tensor(out=OUT[:, 0:Wh], in0=V1B[:, :],
                                            in1=V1[:, 0:Wh], op=MIN)

                    dsty = y[c].rearrange("(p k) w -> p k w", k=K)[:, :, side * Wh:(side + 1) * Wh]
                    srco = OUT[:, :].rearrange("p (k c) -> p k c", c=Wh)
                    nc.sync.dma_start(out=dsty, in_=srco)
    return nc


_RUNNER_CACHE = {}
_SHARDED_CACHE = {}  # (B,C,H,W) -> (sharded_jitted_fn, n_params, n_outs); for benchmarking


def _make_runner(nc, n_cores):
    """Build a reusable jitted SPMD callable for `nc` across `n_cores`
    devices. Mirrors concourse.bass2jax.run_bass_via_pjrt's multi-core path
    but returns the jitted function so repeated kernel() calls don't re-jit."""
    import jax
    from jax.sharding import Mesh, PartitionSpec
    from jax.experimental.shard_map import shard_map
    from concourse import bass2jax
    from concourse.bass2jax import _bass_exec_p, install_neuronx_cc_hook

    install_neuronx_cc_hook()

    partition_name = nc.partition_id_tensor.name if nc.partition_id_tensor else None
    in_names, out_names, out_avals, zero_outs = [], [], [], []
    for alloc in nc.m.functions[0].allocations:
        if not isinstance(alloc, mybir.MemoryLocationSet):
            continue
        name = alloc.memorylocations[0].name
        if alloc.kind == "ExternalInput":
            if name != partition_name:
                in_names.append(name)
        elif alloc.kind == "ExternalOutput":
            shape = tuple(alloc.tensor_shape)
            dtype = mybir.dt.np(alloc.dtype)
            out_names.append(name)
            out_avals.append(jax.core.ShapedArray(shape, dtype))
            zero_outs.append(np.zeros(shape, dtype))
    n_params = len(in_names)
    n_outs = len(out_avals)
    all_in_names = list(in_names) + list(out_names)
    if partition_name is not None:
        all_in_names.append(partition_name)

    def _body(*args):
        operands = list(args)
        if partition_name is not None:
            operands.append(bass2jax.partition_id_tensor())
        outs = _bass_exec_p.bind(
            *operands,
            out_avals=tuple(out_avals),
            in_names=tuple(all_in_names),
            out_names=tuple(out_names),
            lowering_input_output_aliases=(),
            sim_require_finite=True,
            sim_require_nnan=True,
            nc=nc,
        )
        return tuple(outs)

    devices = jax.devices()[:n_cores]
    mesh = Mesh(np.asarray(devices), ("core",))
    in_specs = (PartitionSpec("core"),) * (n_params + n_outs)
    out_specs = (PartitionSpec("core"),) * n_outs
    sharded = jax.jit(
        shard_map(_body, mesh=mesh, in_specs=in_specs, out_specs=out_specs,
                  check_rep=False),
        donate_argnums=tuple(range(n_params, n_params + n_outs)),
        keep_unused=True,
    )
    zshapes = [(n_cores * z.shape[0], *z.shape[1:]) for z in zero_outs]
    zdtypes = [z.dtype for z in zero_outs]

    def run(concat_inputs):
        zeros = [np.zeros(s, d) for s, d in zip(zshapes, zdtypes)]
        return sharded(*concat_inputs, *zeros)

    return run, (sharded, n_params, n_outs)


def kernel(x, m):
    m = int(m)
    assert m == 1, f"kernel hardcodes m=1 (3x3 erosion), got m={m}"
    x = np.ascontiguousarray(np.asarray(x, dtype=np.float32))
    B, C, H, W = x.shape
    assert B == N_CORES, f"batch {B} must equal n_cores {N_CORES}"

    key = (B, C, H, W)
    if key not in _RUNNER_CACHE:
        nc = _build_erosion(C=C, H=H, W=W)
        _split_sync_waits(nc, 1)
        run_, sharded_info = _make_runner(nc, n_cores=B)
        _RUNNER_CACHE[key] = run_
        _SHARDED_CACHE[key] = sharded_info
    run = _RUNNER_CACHE[key]

    # shard batch across cores: per-core input is x[b] of shape (C, H, W);
    # shard_map slices axis 0, so the concatenated input is x reshaped.
    concat = x.reshape(B * C, H, W)
    (out,) = run([concat])
    return np.asarray(out).reshape(B, C, H, W)

